# revision 29
# baseline (speedup 1.0000x reference)
"""Multi-scale patch pooling (gather + segment-mean) for CLIP-AD on 8 trn2 cores.

The reference computes, per batch element b:
    large[b, g, :] = mean over l of tokens[b, large_mask[l, g], :]   (9-elt mean, 169 groups)
    mid[b, g, :]   = mean over l of tokens[b, mid_mask[l, g], :]     (4-elt mean, 196 groups)
    cls[b, 0, :]   = mean over t of tokens[b, t, :]                  (225-elt mean)
    out = concat([large, mid, cls], axis=1)                          # [B, 366, D]

Per batch this is exactly out_b = diag(s) @ W01 @ tokens_b where W01 is a tiny
[366, 225] 0/1 membership-count matrix built host-side from the masks (handles
arbitrary / duplicate indices) and s[g] = 1/group_size. The device kernel runs
the matmul on the tensor engine. For full fp32 accuracy at bf16 matmul speed,
tokens are split host-side into bf16 hi + bf16 lo (x ~= hi + lo to ~2^-18 rel);
W01 entries are small integers, exact in bf16. The per-group 1/L scale is
applied during the PSUM->SBUF evacuation copy.

DMA design (all bulk traffic on gpsimd/SWDGE, whose Q7 descriptor generator
round-robins all 16 SDMA engines; the HWDGE rings were observed to dump whole
transfers on 1-2 engines):
  * Loads: tokens are repacked host-side to [bp/2, 113, 4*2D] so one plain 2D
    DMA with 14336-byte descriptors loads two batches (k-chunk c of batch b
    sits at partition p = token_row - 113*c).
  * Stores: W's columns are permuted so output group 3p+mi lands on PSUM
    partition p of m-tile mi. The per-batch output tile [122, 3*896] is then
    exactly out[b] row-major -> one fully-contiguous store with 10752-byte
    descriptors.

Sharding: pure data parallel on batch — 64 batches per core; weights replicated.
"""

import numpy as np

B, T, D = 512, 225, 896
GL, LL = 169, 9
GM, LM = 196, 4
G = GL + GM + 1  # 366
N_CORES = 8
BP = B // N_CORES  # 64

KP = 113                      # k-chunk partition count (225 -> 113 + 112)
MP = G // 3                   # 122 partitions per m-tile (groups strided by 3)
_K_TILES = ((0, 113), (113, 112))
_N_TILES = ((0, 512), (512, 384))

_CACHE = {}


def _enable_ldw_opt():
    """Flip walrus's hardcoded --enable-ldw-opt=false: consecutive matmuls
    here share their stationary operand 4x, and deduplicating the implicit
    LDWEIGHTS saves ~25% of tensor-engine issue time. Correctness is covered
    by the rel-err check."""
    # Disabled: bacc's move_matmul_waits_to_ldweights emits explicit
    # InstLdweights, which walrus's LDW-opt pass rejects
    # ("InstLdweights is not compatible with LDW optimization").
    return


def _get_nc(bp=BP):
    if bp in _CACHE:
        return _CACHE[bp]
    import concourse.bacc as bacc
    import concourse.mybir as mybir
    import concourse.tile as tile

    _enable_ldw_opt()

    f32 = mybir.dt.float32
    bf16 = mybir.dt.bfloat16

    nc = bacc.Bacc("TRN2", target_bir_lowering=False, debug=False)
    # tokens4[j, p, :] = concat over (bb in 0..1, c in 0..1) of
    #   row(2j+bb, c*113+p), each row = concat(hi, lo) of 2*896 bf16.
    # 32-element (64 B) pad per packed row: keeps the DMA source runs
    # non-contiguous so SWDGE cannot merge them into one stream that it then
    # chunks onto only ~5 of the 16 SDMA engines.
    tokens4 = nc.dram_tensor(
        "tokens4", [bp // 2, KP, 8 * D + 32], bf16, kind="ExternalInput"
    ).ap()
    # w01T_perm[t, mi*122 + p] = W01[3p + mi, t]
    w01T = nc.dram_tensor("w01T", [T, G], bf16, kind="ExternalInput").ap()
    # scale_perm[mi*122 + p] = 1/L of group 3p+mi
    scale = nc.dram_tensor("scale", [G, 1], f32, kind="ExternalInput").ap()
    out = nc.dram_tensor("out", [bp, G, D], f32, kind="ExternalOutput").ap()

    with tile.TileContext(nc) as tc:
        with (
            tc.tile_pool(name="w", bufs=1) as wpool,
            tc.tile_pool(name="tok", bufs=5) as tokpool,
            tc.tile_pool(name="ob", bufs=7) as opool,
            tc.tile_pool(name="ps", bufs=8, space="PSUM") as pspool,
        ):
            # Warm-up ops: the first ACT/DVE instructions pick up extra
            # table-load waits in lowering; give them dummies with no
            # cross-engine deps so real ops keep their wait budget.
            warm = wpool.tile([128, 1], f32, tag="warm")
            nc.gpsimd.memset(warm[:], 0.0)
            nc.scalar.activation(
                warm[:], warm[:], mybir.ActivationFunctionType.Copy
            )
            nc.vector.tensor_copy(warm[:], warm[:])

            w_sb = []
            for ki, (k0, ksz) in enumerate(_K_TILES):
                wt = wpool.tile([128, G], bf16, tag=f"w{ki}")
                nc.gpsimd.dma_start(wt[:ksz, :], w01T[k0 : k0 + ksz, :])
                w_sb.append(wt)
            sc_sb = []
            for mi in range(3):
                st = wpool.tile([128, 1], f32, tag=f"sc{mi}")
                nc.gpsimd.dma_start(st[:MP, :], scale[mi * MP : (mi + 1) * MP, :])
                sc_sb.append(st)

            # Software-pipeline the loads LOOK pairs ahead of compute: the
            # gpsimd Q7 queue is in-order, and a store's wait (on evac) must
            # not block the emission of the next loads.
            LOOK = 3
            npair = bp // 2
            tks = {}

            def emit_load(j):
                tk = tokpool.tile([128, 8 * D], bf16, tag="tok")
                # max_dma_last_dim splits each partition row into 3584B
                # descriptors: SWDGE hands out ~20-25 consecutive descriptors
                # per SDMA engine, so a 113-descriptor transfer only engages
                # ~5 of 16 engines; 452 smaller descriptors engage all 16.
                nc.sync.dma_start(tk[:KP, :], tokens4[j, :, : 8 * D])
                tks[j] = tk

            # Stores are emitted one pair late so their evac-completion waits
            # are already satisfied when the in-order gpsimd queue reaches
            # them — otherwise a waiting store blocks emission of the loads
            # queued behind it and the SDMA engines run dry.
            pending_stores = []

            def flush_stores():
                for dst, src in pending_stores:
                    nc.gpsimd.dma_start(dst, src, max_dma_last_dim=896)
                pending_stores.clear()

            cp = 0
            for j in range(LOOK):
                emit_load(j)
            for j in range(npair):
                if j + LOOK < npair:
                    emit_load(j + LOOK)
                flush_stores()
                tk = tks.pop(j)
                for bb in range(2):
                    b = 2 * j + bb
                    o = opool.tile([128, 3 * D], f32, tag="ob")
                    for mi in range(3):
                        pss = [
                            pspool.tile([128, 512], f32, name="ps", tag="ps")
                            for _ in _N_TILES
                        ]
                        for ki, (k0, ksz) in enumerate(_K_TILES):
                            for part in range(2):  # hi, lo
                                base = bb * 4 * D + ki * 2 * D + part * D
                                for ni, (n0, nsz) in enumerate(_N_TILES):
                                    nc.tensor.matmul(
                                        pss[ni][:MP, :nsz],
                                        w_sb[ki][:ksz, mi * MP : (mi + 1) * MP],
                                        tk[:ksz, base + n0 : base + n0 + nsz],
                                        start=(ki == 0 and part == 0),
                                        stop=(ki == 1 and part == 1),
                                    )
                        # PSUM -> SBUF evacuation with the per-group 1/L
                        # scale. DMA cannot read PSUM; alternate DVE / ACT.
                        for ni, (n0, nsz) in enumerate(_N_TILES):
                            dst = o[:MP, mi * D + n0 : mi * D + n0 + nsz]
                            if cp % 2 == 1:
                                nc.scalar.activation(
                                    dst,
                                    pss[ni][:MP, :nsz],
                                    mybir.ActivationFunctionType.Copy,
                                    scale=sc_sb[mi][:MP, :],
                                )
                            else:
                                nc.vector.tensor_scalar_mul(
                                    dst, pss[ni][:MP, :nsz], sc_sb[mi][:MP, :]
                                )
                            cp += 1
                    # Store: partition p holds groups 3p..3p+2 -> out[b] is
                    # row-major contiguous. Deferred (see pending_stores).
                    pending_stores.append(
                        (out[b].rearrange("(p c) d -> p (c d)", c=3), o[:MP, :])
                    )
            flush_stores()

    nc.compile()
    _CACHE[bp] = nc
    return nc


def _host_prep(tokens_full, large_mask, mid_mask):
    """Split tokens into bf16 hi/lo; build packed tokens, weights, scales."""
    import ml_dtypes

    bf16 = ml_dtypes.bfloat16
    bsz = tokens_full.shape[0]
    hi = tokens_full.astype(bf16)
    lo = (tokens_full - hi.astype(np.float32)).astype(bf16)
    tokens2 = np.concatenate([hi, lo], axis=-1)  # [B, T, 2D] bf16

    # Pack: tokens4[j, p, (bb, c)] = row(2j+bb, c*113+p); row 225 of a batch
    # aliases the next batch's row 0 (junk, partition 112 of chunk 1 unused);
    # one zero pad row covers the very last access.
    flat = np.concatenate(
        [tokens2.reshape(bsz * T, 2 * D), np.zeros((1, 2 * D), bf16)], axis=0
    )
    jj = np.arange(bsz // 2)[:, None, None, None]
    pp = np.arange(KP)[None, :, None, None]
    bb = np.arange(2)[None, None, :, None]
    cc = np.arange(2)[None, None, None, :]
    idx = np.minimum((2 * jj + bb) * T + cc * KP + pp, bsz * T)
    tokens4 = flat[idx].reshape(bsz // 2, KP, 8 * D)
    # 64 B pad per row (see kernel dram decl) to defeat SWDGE run-merging.
    tokens4 = np.concatenate(
        [tokens4, np.zeros((bsz // 2, KP, 32), tokens4.dtype)], axis=2
    )

    W = np.zeros((G, T), np.float32)
    rows = np.arange(GL)
    for l in range(large_mask.shape[0]):
        np.add.at(W, (rows, large_mask[l]), 1.0)
    rows = GL + np.arange(GM)
    for l in range(mid_mask.shape[0]):
        np.add.at(W, (rows, mid_mask[l]), 1.0)
    W[G - 1, :] = 1.0

    s = np.empty(G, np.float32)
    s[:GL] = 1.0 / large_mask.shape[0]
    s[GL : GL + GM] = 1.0 / mid_mask.shape[0]
    s[G - 1] = 1.0 / T

    # Permute groups so m-tile mi, partition p <-> group 3p+mi.
    perm = np.concatenate([np.arange(mi, G, 3) for mi in range(3)])
    w01T = np.ascontiguousarray(W[perm].T).astype(bf16)  # [T, G], ints: exact
    s_perm = np.ascontiguousarray(s[perm].reshape(G, 1))
    return tokens4, w01T, s_perm


def _in_maps(tokens4, w01T, s, n_cores=N_CORES):
    jp = tokens4.shape[0] // n_cores
    return [
        {
            "tokens4": np.ascontiguousarray(tokens4[c * jp : (c + 1) * jp]),
            "w01T": w01T,
            "scale": s,
        }
        for c in range(n_cores)
    ]


def kernel(**inputs):
    from concourse import bass_utils

    tokens_full = np.ascontiguousarray(np.asarray(inputs["patch_tokens"], np.float32))
    large = np.asarray(inputs["large_mask"]).astype(np.int64)
    mid = np.asarray(inputs["mid_mask"]).astype(np.int64)
    tokens4, w01T, s = _host_prep(tokens_full, large, mid)

    nc = _get_nc()
    res = bass_utils.run_bass_kernel_spmd(
        nc, _in_maps(tokens4, w01T, s), core_ids=list(range(N_CORES))
    )
    return np.concatenate(
        [res.results[c]["out"] for c in range(N_CORES)], axis=0
    ).astype(np.float32)


# revision 33
# speedup vs baseline: 2.7186x; 2.7186x over previous
"""Multi-scale patch pooling (gather + segment-mean) for CLIP-AD on 8 trn2 cores.

The reference computes, per batch element b:
    large[b, g, :] = mean over l of tokens[b, large_mask[l, g], :]   (9-elt mean, 169 groups)
    mid[b, g, :]   = mean over l of tokens[b, mid_mask[l, g], :]     (4-elt mean, 196 groups)
    cls[b, 0, :]   = mean over t of tokens[b, t, :]                  (225-elt mean)
    out = concat([large, mid, cls], axis=1)                          # [B, 366, D]

Per batch this is exactly out_b = diag(s) @ W01 @ tokens_b where W01 is a tiny
[366, 225] 0/1 membership-count matrix built host-side from the masks (handles
arbitrary / duplicate indices) and s[g] = 1/group_size. The device kernel runs
the matmul on the tensor engine. For full fp32 accuracy at bf16 matmul speed,
tokens are split host-side into bf16 hi + bf16 lo (x ~= hi + lo to ~2^-18 rel);
W01 entries are small integers, exact in bf16. The per-group 1/L scale is
applied during the PSUM->SBUF evacuation copy.

DMA design (all bulk traffic on gpsimd/SWDGE, whose Q7 descriptor generator
round-robins all 16 SDMA engines; the HWDGE rings were observed to dump whole
transfers on 1-2 engines):
  * Loads: tokens are repacked host-side to [bp/2, 113, 4*2D] so one plain 2D
    DMA with 14336-byte descriptors loads two batches (k-chunk c of batch b
    sits at partition p = token_row - 113*c).
  * Stores: W's columns are permuted so output group 3p+mi lands on PSUM
    partition p of m-tile mi. The per-batch output tile [122, 3*896] is then
    exactly out[b] row-major -> one fully-contiguous store with 10752-byte
    descriptors.

Sharding: pure data parallel on batch — 64 batches per core; weights replicated.
"""

import numpy as np

B, T, D = 512, 225, 896
GL, LL = 169, 9
GM, LM = 196, 4
G = GL + GM + 1  # 366
N_CORES = 8
BP = B // N_CORES  # 64

KP = 113                      # k-chunk partition count (225 -> 113 + 112)
MP = G // 3                   # 122 partitions per m-tile (groups strided by 3)
_K_TILES = ((0, 113), (113, 112))
_N_TILES = ((0, 512), (512, 384))

_CACHE = {}


def _enable_ldw_opt():
    """Flip walrus's hardcoded --enable-ldw-opt=false: consecutive matmuls
    here share their stationary operand 4x, and deduplicating the implicit
    LDWEIGHTS saves ~25% of tensor-engine issue time. Correctness is covered
    by the rel-err check."""
    # Disabled: bacc's move_matmul_waits_to_ldweights emits explicit
    # InstLdweights, which walrus's LDW-opt pass rejects
    # ("InstLdweights is not compatible with LDW optimization").
    return


def _get_nc(bp=BP):
    if bp in _CACHE:
        return _CACHE[bp]
    import concourse.bacc as bacc
    import concourse.mybir as mybir
    import concourse.tile as tile

    _enable_ldw_opt()

    f32 = mybir.dt.float32
    bf16 = mybir.dt.bfloat16

    nc = bacc.Bacc("TRN2", target_bir_lowering=False, debug=False)
    # tokens4[j, p, :] = concat over (bb in 0..1, c in 0..1) of
    #   row(2j+bb, c*113+p), each row = concat(hi, lo) of 2*896 bf16.
    # 32-element (64 B) pad per packed row: keeps the DMA source runs
    # non-contiguous so SWDGE cannot merge them into one stream that it then
    # chunks onto only ~5 of the 16 SDMA engines.
    tokens4 = nc.dram_tensor(
        "tokens4", [bp // 2, KP, 8 * D + 32], bf16, kind="ExternalInput"
    ).ap()
    # w01T_perm[t, mi*122 + p] = W01[3p + mi, t]
    w01T = nc.dram_tensor("w01T", [T, G], bf16, kind="ExternalInput").ap()
    # scale_perm[mi*122 + p] = 1/L of group 3p+mi
    scale = nc.dram_tensor("scale", [G, 1], f32, kind="ExternalInput").ap()
    out = nc.dram_tensor("out", [bp, G, D], f32, kind="ExternalOutput").ap()

    from contextlib import ExitStack

    NTOK = 8  # token slots, spread across SBUF so loads hit distinct engines
    NOB = 6   # output slots, interleaved between them

    with tile.TileContext(nc) as tc:
        with ExitStack() as ctx:
            # SWDGE assigns a DMA's SDMA engines from its write-side address
            # (~14 KB granularity on SBUF). Interleave single-buffer token and
            # output pools so consecutive token slots sit ~25 KB apart -> the
            # in-flight loads drain on different engines instead of queueing
            # on one.
            tok_pools = []
            o_pools = []
            for s in range(NTOK):
                tok_pools.append(
                    ctx.enter_context(tc.tile_pool(name=f"tokp{s}", bufs=1))
                )
                if s < NOB:
                    o_pools.append(
                        ctx.enter_context(tc.tile_pool(name=f"obp{s}", bufs=1))
                    )
            wpool = ctx.enter_context(tc.tile_pool(name="w", bufs=1))
            pspool = ctx.enter_context(
                tc.tile_pool(name="ps", bufs=8, space="PSUM")
            )
            # Warm-up ops: the first ACT/DVE instructions pick up extra
            # table-load waits in lowering; give them dummies with no
            # cross-engine deps so real ops keep their wait budget.
            warm = wpool.tile([128, 1], f32, tag="warm")
            nc.gpsimd.memset(warm[:], 0.0)
            nc.scalar.activation(
                warm[:], warm[:], mybir.ActivationFunctionType.Copy
            )
            nc.vector.tensor_copy(warm[:], warm[:])

            w_sb = []
            for ki, (k0, ksz) in enumerate(_K_TILES):
                wt = wpool.tile([128, G], bf16, tag=f"w{ki}")
                nc.gpsimd.dma_start(wt[:ksz, :], w01T[k0 : k0 + ksz, :])
                w_sb.append(wt)
            sc_sb = []
            for mi in range(3):
                st = wpool.tile([128, 1], f32, tag=f"sc{mi}")
                nc.gpsimd.dma_start(st[:MP, :], scale[mi * MP : (mi + 1) * MP, :])
                sc_sb.append(st)

            # Software-pipeline the loads LOOK pairs ahead of compute: the
            # gpsimd Q7 queue is in-order, and a store's wait (on evac) must
            # not block the emission of the next loads.
            LOOK = 5
            npair = bp // 2
            tks = {}

            def emit_load(j):
                tk = tok_pools[j % NTOK].tile(
                    [128, 8 * D], bf16, name="tok", tag="tok"
                )
                nc.gpsimd.dma_start(tk[:KP, :], tokens4[j, :, : 8 * D])
                tks[j] = tk

            # Stores are emitted one pair late so their evac-completion waits
            # are already satisfied when the in-order gpsimd queue reaches
            # them — otherwise a waiting store blocks emission of the loads
            # queued behind it and the SDMA engines run dry.
            pending_stores = []

            def flush_stores():
                for dst, src in pending_stores:
                    nc.gpsimd.dma_start(dst, src, max_dma_last_dim=896)
                pending_stores.clear()

            cp = 0
            for j in range(LOOK):
                emit_load(j)
            for j in range(npair):
                if j + LOOK < npair:
                    emit_load(j + LOOK)
                flush_stores()
                tk = tks.pop(j)
                for bb in range(2):
                    b = 2 * j + bb
                    o = o_pools[b % NOB].tile(
                        [128, 3 * D], f32, name="ob", tag="ob"
                    )
                    for mi in range(3):
                        pss = [
                            pspool.tile([128, 512], f32, name="ps", tag="ps")
                            for _ in _N_TILES
                        ]
                        for ki, (k0, ksz) in enumerate(_K_TILES):
                            for part in range(2):  # hi, lo
                                base = bb * 4 * D + ki * 2 * D + part * D
                                for ni, (n0, nsz) in enumerate(_N_TILES):
                                    nc.tensor.matmul(
                                        pss[ni][:MP, :nsz],
                                        w_sb[ki][:ksz, mi * MP : (mi + 1) * MP],
                                        tk[:ksz, base + n0 : base + n0 + nsz],
                                        start=(ki == 0 and part == 0),
                                        stop=(ki == 1 and part == 1),
                                    )
                        # PSUM -> SBUF evacuation with the per-group 1/L
                        # scale. DMA cannot read PSUM; alternate DVE / ACT.
                        for ni, (n0, nsz) in enumerate(_N_TILES):
                            dst = o[:MP, mi * D + n0 : mi * D + n0 + nsz]
                            if cp % 2 == 1:
                                nc.scalar.activation(
                                    dst,
                                    pss[ni][:MP, :nsz],
                                    mybir.ActivationFunctionType.Copy,
                                    scale=sc_sb[mi][:MP, :],
                                )
                            else:
                                nc.vector.tensor_scalar_mul(
                                    dst, pss[ni][:MP, :nsz], sc_sb[mi][:MP, :]
                                )
                            cp += 1
                    # Store: partition p holds groups 3p..3p+2 -> out[b] is
                    # row-major contiguous. Deferred (see pending_stores).
                    pending_stores.append(
                        (out[b].rearrange("(p c) d -> p (c d)", c=3), o[:MP, :])
                    )
            flush_stores()

    nc.compile()
    _CACHE[bp] = nc
    return nc


def _host_prep(tokens_full, large_mask, mid_mask):
    """Split tokens into bf16 hi/lo; build packed tokens, weights, scales."""
    import ml_dtypes

    bf16 = ml_dtypes.bfloat16
    bsz = tokens_full.shape[0]
    hi = tokens_full.astype(bf16)
    lo = (tokens_full - hi.astype(np.float32)).astype(bf16)
    tokens2 = np.concatenate([hi, lo], axis=-1)  # [B, T, 2D] bf16

    # Pack: tokens4[j, p, (bb, c)] = row(2j+bb, c*113+p); row 225 of a batch
    # aliases the next batch's row 0 (junk, partition 112 of chunk 1 unused);
    # one zero pad row covers the very last access.
    flat = np.concatenate(
        [tokens2.reshape(bsz * T, 2 * D), np.zeros((1, 2 * D), bf16)], axis=0
    )
    jj = np.arange(bsz // 2)[:, None, None, None]
    pp = np.arange(KP)[None, :, None, None]
    bb = np.arange(2)[None, None, :, None]
    cc = np.arange(2)[None, None, None, :]
    idx = np.minimum((2 * jj + bb) * T + cc * KP + pp, bsz * T)
    tokens4 = flat[idx].reshape(bsz // 2, KP, 8 * D)
    # 64 B pad per row (see kernel dram decl) to defeat SWDGE run-merging.
    tokens4 = np.concatenate(
        [tokens4, np.zeros((bsz // 2, KP, 32), tokens4.dtype)], axis=2
    )

    W = np.zeros((G, T), np.float32)
    rows = np.arange(GL)
    for l in range(large_mask.shape[0]):
        np.add.at(W, (rows, large_mask[l]), 1.0)
    rows = GL + np.arange(GM)
    for l in range(mid_mask.shape[0]):
        np.add.at(W, (rows, mid_mask[l]), 1.0)
    W[G - 1, :] = 1.0

    s = np.empty(G, np.float32)
    s[:GL] = 1.0 / large_mask.shape[0]
    s[GL : GL + GM] = 1.0 / mid_mask.shape[0]
    s[G - 1] = 1.0 / T

    # Permute groups so m-tile mi, partition p <-> group 3p+mi.
    perm = np.concatenate([np.arange(mi, G, 3) for mi in range(3)])
    w01T = np.ascontiguousarray(W[perm].T).astype(bf16)  # [T, G], ints: exact
    s_perm = np.ascontiguousarray(s[perm].reshape(G, 1))
    return tokens4, w01T, s_perm


def _in_maps(tokens4, w01T, s, n_cores=N_CORES):
    jp = tokens4.shape[0] // n_cores
    return [
        {
            "tokens4": np.ascontiguousarray(tokens4[c * jp : (c + 1) * jp]),
            "w01T": w01T,
            "scale": s,
        }
        for c in range(n_cores)
    ]


def kernel(**inputs):
    from concourse import bass_utils

    tokens_full = np.ascontiguousarray(np.asarray(inputs["patch_tokens"], np.float32))
    large = np.asarray(inputs["large_mask"]).astype(np.int64)
    mid = np.asarray(inputs["mid_mask"]).astype(np.int64)
    tokens4, w01T, s = _host_prep(tokens_full, large, mid)

    nc = _get_nc()
    res = bass_utils.run_bass_kernel_spmd(
        nc, _in_maps(tokens4, w01T, s), core_ids=list(range(N_CORES))
    )
    return np.concatenate(
        [res.results[c]["out"] for c in range(N_CORES)], axis=0
    ).astype(np.float32)
